# revision 5
# baseline (speedup 1.0000x reference)
"""Trainium2 Bass kernel for CustomMLP: out = GELU(x@W1+b1)@W2 + b2.

x: (4, 2048, 1024) f32, W1: (1024, 4096), b1: (4096,), W2: (4096, 1024),
b2: (1024,). Data-parallel over the 8192 flattened rows: each of the 8
NeuronCores handles 1024 rows with fully replicated weights (no
collectives).

Per-core layout (everything transposed so both matmuls contract on the
partition axis with no on-chip transposes):
  xT   [1024(e), 1024(m)]  bf16    = x_shard^T
  hT   [h, m] bf16 on chip         (GELU applied on PSUM eviction)
  outT [1024(e2), 1024(m)] f32     host transposes back

matmul1: psum[h_blk, m] += w1[e_blk, h_blk].T @ xT[e_blk, m]
matmul2: psum[e2_blk, m] += w2[h_blk, e2_blk].T @ hT[h_blk, m]

All tensor data is bf16 (PSUM accumulates f32): halves HBM traffic vs
f32 and enables the PE's Fast Weight Load path (fp32 disables FWL),
hiding LDWEIGHTS behind the moving stream. Weights stream on the
scalar-engine HWDGE ring; xT/out use the sync-engine ring, so the two
flows don't serialize behind each other.

Weights are host-packed so every DMA lands 2KB-contiguous per partition.
"""
import numpy as np
import ml_dtypes

import concourse.bass as bass
import concourse.mybir as mybir
import concourse.tile as tile
from concourse import bacc
from concourse.bass_utils import run_bass_kernel_spmd

P = 128
N_CORES = 8

F32 = mybir.dt.float32
BF16 = mybir.dt.bfloat16
NP_BF16 = ml_dtypes.bfloat16
GELU = mybir.ActivationFunctionType.Gelu
IDENT = mybir.ActivationFunctionType.Identity


def build_nc(M=1024, E=1024, H=4096, E2=1024, mm_dtype=BF16, act=GELU):
    """Build + compile the per-core program. M/E/H/E2 parameterized so a
    scaled-down version can run in CoreSim."""
    EB, HB, E2B = E // P, H // P, E2 // P
    MH = max(1, M // 512)  # m halves (moving-dim chunks of <=512)
    MS = M // MH           # moving chunk size
    W2Q = min(8, HB)       # h-blocks per w2 slab
    NQ = HB // W2Q         # w2 slabs per e2 block

    mmdt = mm_dtype
    nc = bacc.Bacc(None, target_bir_lowering=False)
    xT_d = nc.declare_dram_parameter("xT", [E, M], mmdt, isOutput=False)
    w1_d = nc.declare_dram_parameter("w1p", [HB, P, EB, P], mmdt, isOutput=False)
    b1_d = nc.declare_dram_parameter("b1p", [P, HB], F32, isOutput=False)
    w2_d = nc.declare_dram_parameter("w2p", [E2B, P, HB, P], mmdt, isOutput=False)
    b2_d = nc.declare_dram_parameter("b2p", [P, E2B], F32, isOutput=False)
    out_d = nc.declare_dram_parameter("outT", [E2B, P, M], F32, isOutput=True)

    xT_v = xT_d.rearrange("(eb p) m -> p eb m", p=P)

    with tile.TileContext(nc) as tc:
        with (
            tc.tile_pool(name="const", bufs=1) as cpool,
            tc.tile_pool(name="xp", bufs=1) as xpool,
            tc.tile_pool(name="hp", bufs=1) as hpool,
            tc.tile_pool(name="w1p", bufs=5) as w1pool,
            tc.tile_pool(name="w2p", bufs=4) as w2pool,
            tc.tile_pool(name="op", bufs=2) as opool,
            tc.tile_pool(name="ps1", bufs=4, space="PSUM") as psum1,
            tc.tile_pool(name="ps2", bufs=4, space="PSUM") as psum2,
        ):
            b1_sb = cpool.tile([P, HB], F32, name="b1s")
            b2_sb = cpool.tile([P, E2B], F32, name="b2s")

            # Weights ride the scalar-engine HWDGE ring; xT/out ride the
            # sync ring. Each DMA issue costs ~650ns on the issuing
            # engine and only 4 can be outstanding per queue, so batch
            # into few, large transfers; the one exception is a small
            # xT eb0 chunk up front so the first matmul starts ASAP.
            HEAD_HBS = min(4, HB)
            w1_tiles = {}
            w1_tiles[0] = w1pool.tile([P, EB, P], mmdt, name="w1t")
            nc.scalar.dma_start(out=w1_tiles[0][:], in_=w1_d[0])

            xT_sb = xpool.tile([P, EB, M], mmdt, name="xT")
            nc.sync.dma_start(out=xT_sb[:, 0, 0:MS], in_=xT_v[:, 0, 0:MS])
            if EB > 1:
                nc.sync.dma_start(
                    out=xT_sb[:, 1:EB, 0:MS], in_=xT_v[:, 1:EB, 0:MS]
                )
            for hb in range(1, HEAD_HBS):
                w1_tiles[hb] = w1pool.tile([P, EB, P], mmdt, name="w1t")
                nc.scalar.dma_start(out=w1_tiles[hb][:], in_=w1_d[hb])
            nc.scalar.dma_start(out=b1_sb[:], in_=b1_d[:])
            nc.scalar.dma_start(out=b2_sb[:], in_=b2_d[:])
            for mh in range(1, MH):
                ms = slice(mh * MS, (mh + 1) * MS)
                nc.sync.dma_start(out=xT_sb[:, :, ms], in_=xT_v[:, :, ms])

            hT_sb = hpool.tile([P, HB, M], mmdt, name="hT")

            def mm1_group(w1_t, hb, mh):
                ms = slice(mh * MS, (mh + 1) * MS)
                ps = psum1.tile([P, MS], F32, name="ps1")
                for eb in range(EB):
                    nc.tensor.matmul(
                        ps[:],
                        lhsT=w1_t[:, eb, :],
                        rhs=xT_sb[:, eb, ms],
                        start=(eb == 0),
                        stop=(eb == EB - 1),
                    )
                nc.scalar.activation(
                    hT_sb[:, hb, ms], ps[:], act, bias=b1_sb[:, hb : hb + 1]
                )

            # ---- matmul 1 + GELU ----
            for mh in range(MH):
                for hb in range(HEAD_HBS):
                    mm1_group(w1_tiles[hb], hb, mh)
            for hb in range(HEAD_HBS, HB):
                w1_t = w1pool.tile([P, EB, P], mmdt, name="w1t")
                nc.scalar.dma_start(out=w1_t[:], in_=w1_d[hb])
                for mh in range(MH):
                    mm1_group(w1_t, hb, mh)

            # ---- matmul 2 + bias ----
            for e2b in range(E2B):
                w2_ts = []
                for q in range(NQ):
                    w2_t = w2pool.tile([P, W2Q, P], mmdt, name="w2t")
                    nc.scalar.dma_start(
                        out=w2_t[:], in_=w2_d[e2b, :, q * W2Q : (q + 1) * W2Q, :]
                    )
                    w2_ts.append(w2_t)
                out_sb = opool.tile([P, M], F32, name="outsb")
                for mh in range(MH):
                    ms = slice(mh * MS, (mh + 1) * MS)
                    ps2 = psum2.tile([P, MS], F32, name="ps2")
                    for hb in range(HB):
                        nc.tensor.matmul(
                            ps2[:],
                            lhsT=w2_ts[hb // W2Q][:, hb % W2Q, :],
                            rhs=hT_sb[:, hb, ms],
                            start=(hb == 0),
                            stop=(hb == HB - 1),
                        )
                    # For the very last group, evict+store in small
                    # slices so the final out DMA trails the final
                    # matmul by as little as possible.
                    last = e2b == E2B - 1 and mh == MH - 1
                    nsl = 4 if last else 1
                    sw = MS // nsl
                    for sl in range(nsl):
                        mss = slice(mh * MS + sl * sw, mh * MS + (sl + 1) * sw)
                        pss = slice(sl * sw, (sl + 1) * sw)
                        nc.scalar.activation(
                            out_sb[:, mss],
                            ps2[:, pss],
                            IDENT,
                            bias=b2_sb[:, e2b : e2b + 1],
                        )
                        nc.sync.dma_start(
                            out=out_d[e2b, :, mss], in_=out_sb[:, mss]
                        )

    nc.compile()
    return nc


def pack_inputs(x, w1, b1, w2, b2):
    """Host-side shard + pack. Returns per-core input maps."""
    M_TOT = x.shape[0] * x.shape[1]
    E = x.shape[2]
    H = w1.shape[1]
    E2 = w2.shape[1]
    MC = M_TOT // N_CORES
    xf = np.ascontiguousarray(x.reshape(M_TOT, E))

    w1p = np.ascontiguousarray(
        w1.reshape(E // P, P, H // P, P).transpose(2, 1, 0, 3)
    ).astype(NP_BF16)
    w2p = np.ascontiguousarray(
        w2.reshape(H // P, P, E2 // P, P).transpose(2, 1, 0, 3)
    ).astype(NP_BF16)
    b1p = np.ascontiguousarray(b1.reshape(H // P, P).T)
    b2p = np.ascontiguousarray(b2.reshape(E2 // P, P).T)

    in_maps = []
    for i in range(N_CORES):
        xTi = np.ascontiguousarray(xf[i * MC : (i + 1) * MC].T).astype(NP_BF16)
        in_maps.append(
            {"xT": xTi, "w1p": w1p, "b1p": b1p, "w2p": w2p, "b2p": b2p}
        )
    return in_maps


def unpack_outputs(results, batch_shape=(4, 2048), E2=1024):
    M_TOT = batch_shape[0] * batch_shape[1]
    MC = M_TOT // N_CORES
    out = np.empty((M_TOT, E2), dtype=np.float32)
    for i in range(N_CORES):
        o = results[i]["outT"]  # [E2B, P, MC]
        out[i * MC : (i + 1) * MC] = o.transpose(2, 0, 1).reshape(MC, E2)
    return out.reshape(*batch_shape, E2)


_NC_CACHE = {}


def _get_nc():
    if "nc" not in _NC_CACHE:
        _NC_CACHE["nc"] = build_nc()
    return _NC_CACHE["nc"]


def kernel(x, w1, b1, w2, b2):
    nc = _get_nc()
    in_maps = pack_inputs(
        np.asarray(x, dtype=np.float32),
        np.asarray(w1, dtype=np.float32),
        np.asarray(b1, dtype=np.float32),
        np.asarray(w2, dtype=np.float32),
        np.asarray(b2, dtype=np.float32),
    )
    res = run_bass_kernel_spmd(nc, in_maps, core_ids=list(range(N_CORES))).results
    return unpack_outputs(res, batch_shape=(x.shape[0], x.shape[1]), E2=w2.shape[1])


# revision 8
# speedup vs baseline: 1.0193x; 1.0193x over previous
"""Trainium2 Bass kernel for CustomMLP: out = GELU(x@W1+b1)@W2 + b2.

x: (4, 2048, 1024) f32, W1: (1024, 4096), b1: (4096,), W2: (4096, 1024),
b2: (1024,). Data-parallel over the 8192 flattened rows: each of the 8
NeuronCores handles 1024 rows with fully replicated weights (no
collectives).

Per-core layout (everything transposed so both matmuls contract on the
partition axis with no on-chip transposes):
  xT   [1024(e), 1024(m)]  bf16    = x_shard^T
  hT   [h, m] bf16 on chip         (GELU applied on PSUM eviction)
  outT [1024(e2), 1024(m)] f32     host transposes back

matmul1: psum[h_blk, m] += w1[e_blk, h_blk].T @ xT[e_blk, m]
matmul2: psum[e2_blk, m] += w2[h_blk, e2_blk].T @ hT[h_blk, m]

All tensor data is bf16 (PSUM accumulates f32): halves HBM traffic vs
f32 and enables the PE's Fast Weight Load path (fp32 disables FWL),
hiding LDWEIGHTS behind the moving stream. Weights stream on the
scalar-engine HWDGE ring; xT/out use the sync-engine ring, so the two
flows don't serialize behind each other.

Weights are host-packed so every DMA lands 2KB-contiguous per partition.
"""
import numpy as np
import ml_dtypes

import concourse.bass as bass
import concourse.mybir as mybir
import concourse.tile as tile
from concourse import bacc
from concourse.bass_utils import run_bass_kernel_spmd

P = 128
N_CORES = 8

F32 = mybir.dt.float32
BF16 = mybir.dt.bfloat16
NP_BF16 = ml_dtypes.bfloat16
GELU = mybir.ActivationFunctionType.Gelu
IDENT = mybir.ActivationFunctionType.Identity


def build_nc(M=1024, E=1024, H=4096, E2=1024, mm_dtype=BF16, act=GELU):
    """Build + compile the per-core program. M/E/H/E2 parameterized so a
    scaled-down version can run in CoreSim."""
    EB, HB, E2B = E // P, H // P, E2 // P
    MH = max(1, M // 512)  # m halves (moving-dim chunks of <=512)
    MS = M // MH           # moving chunk size
    W2Q = min(8, HB)       # h-blocks per w2 slab
    NQ = HB // W2Q         # w2 slabs per e2 block

    mmdt = mm_dtype
    nc = bacc.Bacc(None, target_bir_lowering=False)
    xT_d = nc.declare_dram_parameter("xT", [E, M], mmdt, isOutput=False)
    w1_d = nc.declare_dram_parameter("w1p", [HB, P, EB, P], mmdt, isOutput=False)
    b1_d = nc.declare_dram_parameter("b1p", [P, HB], F32, isOutput=False)
    w2_d = nc.declare_dram_parameter("w2p", [E2B, P, HB, P], mmdt, isOutput=False)
    b2_d = nc.declare_dram_parameter("b2p", [P, E2B], F32, isOutput=False)
    out_d = nc.declare_dram_parameter("outT", [E2B, P, M], F32, isOutput=True)

    xT_v = xT_d.rearrange("(eb p) m -> p eb m", p=P)

    with tile.TileContext(nc) as tc:
        with (
            tc.tile_pool(name="const", bufs=1) as cpool,
            tc.tile_pool(name="xp", bufs=1) as xpool,
            tc.tile_pool(name="hp", bufs=1) as hpool,
            tc.tile_pool(name="w1p", bufs=5) as w1pool,
            tc.tile_pool(name="w2p", bufs=8) as w2pool,
            tc.tile_pool(name="op", bufs=2) as opool,
            tc.tile_pool(name="ps1", bufs=4, space="PSUM") as psum1,
            tc.tile_pool(name="ps2", bufs=4, space="PSUM") as psum2,
        ):
            b1_sb = cpool.tile([P, HB], F32, name="b1s")
            b2_sb = cpool.tile([P, E2B], F32, name="b2s")

            # Weights ride the scalar-engine HWDGE ring; xT/out ride the
            # sync ring. Each DMA issue costs ~650ns on the issuing
            # engine and only 4 can be outstanding per queue, so batch
            # into few, large transfers; the one exception is a small
            # xT eb0 chunk up front so the first matmul starts ASAP.
            HEAD_HBS = min(4, HB)
            w1_tiles = {}
            w1_tiles[0] = w1pool.tile([P, EB, P], mmdt, name="w1t")
            nc.scalar.dma_start(out=w1_tiles[0][:], in_=w1_d[0])

            xT_sb = xpool.tile([P, EB, M], mmdt, name="xT")
            nc.sync.dma_start(out=xT_sb[:, 0, 0:MS], in_=xT_v[:, 0, 0:MS])
            eb_mid = min(4, EB)
            if EB > 1:
                nc.sync.dma_start(
                    out=xT_sb[:, 1:eb_mid, 0:MS], in_=xT_v[:, 1:eb_mid, 0:MS]
                )
            if EB > eb_mid:
                nc.sync.dma_start(
                    out=xT_sb[:, eb_mid:EB, 0:MS], in_=xT_v[:, eb_mid:EB, 0:MS]
                )
            w1_tiles[1] = w1pool.tile([P, EB, P], mmdt, name="w1t")
            nc.scalar.dma_start(out=w1_tiles[1][:], in_=w1_d[1])
            nc.scalar.dma_start(out=b1_sb[:], in_=b1_d[:])
            for hb in range(2, HEAD_HBS):
                w1_tiles[hb] = w1pool.tile([P, EB, P], mmdt, name="w1t")
                nc.scalar.dma_start(out=w1_tiles[hb][:], in_=w1_d[hb])
            nc.scalar.dma_start(out=b2_sb[:], in_=b2_d[:])
            for mh in range(1, MH):
                ms = slice(mh * MS, (mh + 1) * MS)
                nc.sync.dma_start(
                    out=xT_sb[:, 0:eb_mid, ms], in_=xT_v[:, 0:eb_mid, ms]
                )
                if EB > eb_mid:
                    nc.sync.dma_start(
                        out=xT_sb[:, eb_mid:EB, ms], in_=xT_v[:, eb_mid:EB, ms]
                    )

            hT_sb = hpool.tile([P, HB, M], mmdt, name="hT")

            def mm1_group(w1_t, hb, mh):
                ms = slice(mh * MS, (mh + 1) * MS)
                ps = psum1.tile([P, MS], F32, name="ps1")
                for eb in range(EB):
                    nc.tensor.matmul(
                        ps[:],
                        lhsT=w1_t[:, eb, :],
                        rhs=xT_sb[:, eb, ms],
                        start=(eb == 0),
                        stop=(eb == EB - 1),
                    )
                nc.scalar.activation(
                    hT_sb[:, hb, ms], ps[:], act, bias=b1_sb[:, hb : hb + 1]
                )

            # ---- matmul 1 + GELU ----
            for mh in range(MH):
                for hb in range(HEAD_HBS):
                    mm1_group(w1_tiles[hb], hb, mh)
            for hb in range(HEAD_HBS, HB):
                w1_t = w1pool.tile([P, EB, P], mmdt, name="w1t")
                nc.scalar.dma_start(out=w1_t[:], in_=w1_d[hb])
                for mh in range(MH):
                    mm1_group(w1_t, hb, mh)

            # ---- matmul 2 + bias ----
            for e2b in range(E2B):
                w2_ts = []
                for q in range(NQ):
                    w2_t = w2pool.tile([P, W2Q, P], mmdt, name="w2t")
                    nc.scalar.dma_start(
                        out=w2_t[:], in_=w2_d[e2b, :, q * W2Q : (q + 1) * W2Q, :]
                    )
                    w2_ts.append(w2_t)
                out_sb = opool.tile([P, M], F32, name="outsb")
                for mh in range(MH):
                    ms = slice(mh * MS, (mh + 1) * MS)
                    ps2 = psum2.tile([P, MS], F32, name="ps2")
                    for hb in range(HB):
                        nc.tensor.matmul(
                            ps2[:],
                            lhsT=w2_ts[hb // W2Q][:, hb % W2Q, :],
                            rhs=hT_sb[:, hb, ms],
                            start=(hb == 0),
                            stop=(hb == HB - 1),
                        )
                    nc.scalar.activation(
                        out_sb[:, ms], ps2[:], IDENT, bias=b2_sb[:, e2b : e2b + 1]
                    )
                    nc.sync.dma_start(out=out_d[e2b, :, ms], in_=out_sb[:, ms])

    nc.compile()
    return nc


def pack_inputs(x, w1, b1, w2, b2):
    """Host-side shard + pack. Returns per-core input maps."""
    M_TOT = x.shape[0] * x.shape[1]
    E = x.shape[2]
    H = w1.shape[1]
    E2 = w2.shape[1]
    MC = M_TOT // N_CORES
    xf = np.ascontiguousarray(x.reshape(M_TOT, E))

    w1p = np.ascontiguousarray(
        w1.reshape(E // P, P, H // P, P).transpose(2, 1, 0, 3)
    ).astype(NP_BF16)
    w2p = np.ascontiguousarray(
        w2.reshape(H // P, P, E2 // P, P).transpose(2, 1, 0, 3)
    ).astype(NP_BF16)
    b1p = np.ascontiguousarray(b1.reshape(H // P, P).T)
    b2p = np.ascontiguousarray(b2.reshape(E2 // P, P).T)

    in_maps = []
    for i in range(N_CORES):
        xTi = np.ascontiguousarray(xf[i * MC : (i + 1) * MC].T).astype(NP_BF16)
        in_maps.append(
            {"xT": xTi, "w1p": w1p, "b1p": b1p, "w2p": w2p, "b2p": b2p}
        )
    return in_maps


def unpack_outputs(results, batch_shape=(4, 2048), E2=1024):
    M_TOT = batch_shape[0] * batch_shape[1]
    MC = M_TOT // N_CORES
    out = np.empty((M_TOT, E2), dtype=np.float32)
    for i in range(N_CORES):
        o = results[i]["outT"]  # [E2B, P, MC]
        out[i * MC : (i + 1) * MC] = o.transpose(2, 0, 1).reshape(MC, E2)
    return out.reshape(*batch_shape, E2)


_NC_CACHE = {}


def _get_nc():
    if "nc" not in _NC_CACHE:
        _NC_CACHE["nc"] = build_nc()
    return _NC_CACHE["nc"]


def kernel(x, w1, b1, w2, b2):
    nc = _get_nc()
    in_maps = pack_inputs(
        np.asarray(x, dtype=np.float32),
        np.asarray(w1, dtype=np.float32),
        np.asarray(b1, dtype=np.float32),
        np.asarray(w2, dtype=np.float32),
        np.asarray(b2, dtype=np.float32),
    )
    res = run_bass_kernel_spmd(nc, in_maps, core_ids=list(range(N_CORES))).results
    return unpack_outputs(res, batch_shape=(x.shape[0], x.shape[1]), E2=w2.shape[1])


# revision 9
# speedup vs baseline: 1.0257x; 1.0063x over previous
"""Trainium2 Bass kernel for CustomMLP: out = GELU(x@W1+b1)@W2 + b2.

x: (4, 2048, 1024) f32, W1: (1024, 4096), b1: (4096,), W2: (4096, 1024),
b2: (1024,). Data-parallel over the 8192 flattened rows: each of the 8
NeuronCores handles 1024 rows with fully replicated weights (no
collectives).

Per-core layout (everything transposed so both matmuls contract on the
partition axis with no on-chip transposes):
  xT   [1024(e), 1024(m)]  bf16    = x_shard^T
  hT   [h, m] bf16 on chip         (GELU applied on PSUM eviction)
  outT [1024(e2), 1024(m)] f32     host transposes back

matmul1: psum[h_blk, m] += w1[e_blk, h_blk].T @ xT[e_blk, m]
matmul2: psum[e2_blk, m] += w2[h_blk, e2_blk].T @ hT[h_blk, m]

All tensor data is bf16 (PSUM accumulates f32): halves HBM traffic vs
f32 and enables the PE's Fast Weight Load path (fp32 disables FWL),
hiding LDWEIGHTS behind the moving stream. Weights stream on the
scalar-engine HWDGE ring; xT/out use the sync-engine ring, so the two
flows don't serialize behind each other.

Weights are host-packed so every DMA lands 2KB-contiguous per partition.
"""
import numpy as np
import ml_dtypes

import concourse.bass as bass
import concourse.mybir as mybir
import concourse.tile as tile
from concourse import bacc
from concourse.bass_utils import run_bass_kernel_spmd

P = 128
N_CORES = 8

F32 = mybir.dt.float32
BF16 = mybir.dt.bfloat16
NP_BF16 = ml_dtypes.bfloat16
GELU = mybir.ActivationFunctionType.Gelu
IDENT = mybir.ActivationFunctionType.Identity


def build_nc(M=1024, E=1024, H=4096, E2=1024, mm_dtype=BF16, act=GELU):
    """Build + compile the per-core program. M/E/H/E2 parameterized so a
    scaled-down version can run in CoreSim."""
    EB, HB, E2B = E // P, H // P, E2 // P
    MH = max(1, M // 512)  # m halves (moving-dim chunks of <=512)
    MS = M // MH           # moving chunk size
    W2Q = min(8, HB)       # h-blocks per w2 slab
    NQ = HB // W2Q         # w2 slabs per e2 block

    mmdt = mm_dtype
    nc = bacc.Bacc(None, target_bir_lowering=False)
    xT_d = nc.declare_dram_parameter("xT", [E, M], mmdt, isOutput=False)
    w1_d = nc.declare_dram_parameter("w1p", [HB, P, EB, P], mmdt, isOutput=False)
    b1_d = nc.declare_dram_parameter("b1p", [P, HB], F32, isOutput=False)
    w2_d = nc.declare_dram_parameter("w2p", [E2B, P, HB, P], mmdt, isOutput=False)
    b2_d = nc.declare_dram_parameter("b2p", [P, E2B], F32, isOutput=False)
    out_d = nc.declare_dram_parameter("outT", [E2B, P, M], F32, isOutput=True)

    xT_v = xT_d.rearrange("(eb p) m -> p eb m", p=P)

    with tile.TileContext(nc) as tc:
        with (
            tc.tile_pool(name="const", bufs=1) as cpool,
            tc.tile_pool(name="xp", bufs=1) as xpool,
            tc.tile_pool(name="hp", bufs=1) as hpool,
            tc.tile_pool(name="w1p", bufs=5) as w1pool,
            tc.tile_pool(name="w2p", bufs=8) as w2pool,
            tc.tile_pool(name="op", bufs=2) as opool,
            tc.tile_pool(name="ps1", bufs=4, space="PSUM") as psum1,
            tc.tile_pool(name="ps2", bufs=4, space="PSUM") as psum2,
        ):
            b1_sb = cpool.tile([P, HB], F32, name="b1s")
            b2_sb = cpool.tile([P, E2B], F32, name="b2s")

            # PE warmup: ~12 matmuls on scratch data (no DMA deps) run
            # in the otherwise-idle window while the first real tiles
            # stream in, walking the PE clock up its pstate ramp
            # (0.65->1.2->2.4GHz over ~3us of execution) so real work
            # starts at full speed.
            warm_sb = cpool.tile([P, P + 512], mmdt, name="warm")
            nc.gpsimd.memset(warm_sb[:], 0.0)
            for _ in range(12):
                wps = psum1.tile([P, 512], F32, name="ps1")
                nc.tensor.matmul(
                    wps[:],
                    lhsT=warm_sb[:, 0:P],
                    rhs=warm_sb[:, P : P + 512],
                    start=True,
                    stop=True,
                )

            # Weights ride the scalar-engine HWDGE ring; xT/out ride the
            # sync ring. Each DMA issue costs ~650ns on the issuing
            # engine and only 4 can be outstanding per queue, so batch
            # into few, large transfers. First xT half is split across
            # BOTH rings so group 0's data outruns the PE (the DMA
            # completion semaphore fires ~2us after the last byte).
            HEAD_HBS = min(4, HB)
            eb_mid = (EB + 1) // 2
            xT_sb = xpool.tile([P, EB, M], mmdt, name="xT")
            nc.sync.dma_start(out=xT_sb[:, 0, 0:MS], in_=xT_v[:, 0, 0:MS])
            if EB > 1:
                nc.sync.dma_start(
                    out=xT_sb[:, 1:eb_mid, 0:MS], in_=xT_v[:, 1:eb_mid, 0:MS]
                )
            if EB > eb_mid:
                nc.scalar.dma_start(
                    out=xT_sb[:, eb_mid:EB, 0:MS], in_=xT_v[:, eb_mid:EB, 0:MS]
                )
            w1_tiles = {}
            w1_tiles[0] = w1pool.tile([P, EB, P], mmdt, name="w1t")
            nc.scalar.dma_start(out=w1_tiles[0][:], in_=w1_d[0])
            w1_tiles[1] = w1pool.tile([P, EB, P], mmdt, name="w1t")
            nc.scalar.dma_start(out=w1_tiles[1][:], in_=w1_d[1])
            nc.scalar.dma_start(out=b1_sb[:], in_=b1_d[:])
            for hb in range(2, HEAD_HBS):
                w1_tiles[hb] = w1pool.tile([P, EB, P], mmdt, name="w1t")
                nc.scalar.dma_start(out=w1_tiles[hb][:], in_=w1_d[hb])
            nc.scalar.dma_start(out=b2_sb[:], in_=b2_d[:])
            for mh in range(1, MH):
                ms = slice(mh * MS, (mh + 1) * MS)
                nc.sync.dma_start(
                    out=xT_sb[:, 0:eb_mid, ms], in_=xT_v[:, 0:eb_mid, ms]
                )
                if EB > eb_mid:
                    nc.sync.dma_start(
                        out=xT_sb[:, eb_mid:EB, ms], in_=xT_v[:, eb_mid:EB, ms]
                    )

            hT_sb = hpool.tile([P, HB, M], mmdt, name="hT")

            def mm1_group(w1_t, hb, mh):
                ms = slice(mh * MS, (mh + 1) * MS)
                ps = psum1.tile([P, MS], F32, name="ps1")
                for eb in range(EB):
                    nc.tensor.matmul(
                        ps[:],
                        lhsT=w1_t[:, eb, :],
                        rhs=xT_sb[:, eb, ms],
                        start=(eb == 0),
                        stop=(eb == EB - 1),
                    )
                nc.scalar.activation(
                    hT_sb[:, hb, ms], ps[:], act, bias=b1_sb[:, hb : hb + 1]
                )

            # ---- matmul 1 + GELU ----
            for mh in range(MH):
                for hb in range(HEAD_HBS):
                    mm1_group(w1_tiles[hb], hb, mh)
            for hb in range(HEAD_HBS, HB):
                w1_t = w1pool.tile([P, EB, P], mmdt, name="w1t")
                nc.scalar.dma_start(out=w1_t[:], in_=w1_d[hb])
                for mh in range(MH):
                    mm1_group(w1_t, hb, mh)

            # ---- matmul 2 + bias ----
            for e2b in range(E2B):
                w2_ts = []
                for q in range(NQ):
                    w2_t = w2pool.tile([P, W2Q, P], mmdt, name="w2t")
                    nc.scalar.dma_start(
                        out=w2_t[:], in_=w2_d[e2b, :, q * W2Q : (q + 1) * W2Q, :]
                    )
                    w2_ts.append(w2_t)
                out_sb = opool.tile([P, M], F32, name="outsb")
                for mh in range(MH):
                    ms = slice(mh * MS, (mh + 1) * MS)
                    ps2 = psum2.tile([P, MS], F32, name="ps2")
                    for hb in range(HB):
                        nc.tensor.matmul(
                            ps2[:],
                            lhsT=w2_ts[hb // W2Q][:, hb % W2Q, :],
                            rhs=hT_sb[:, hb, ms],
                            start=(hb == 0),
                            stop=(hb == HB - 1),
                        )
                    nc.scalar.activation(
                        out_sb[:, ms], ps2[:], IDENT, bias=b2_sb[:, e2b : e2b + 1]
                    )
                    nc.sync.dma_start(out=out_d[e2b, :, ms], in_=out_sb[:, ms])

    nc.compile()
    return nc


def pack_inputs(x, w1, b1, w2, b2):
    """Host-side shard + pack. Returns per-core input maps."""
    M_TOT = x.shape[0] * x.shape[1]
    E = x.shape[2]
    H = w1.shape[1]
    E2 = w2.shape[1]
    MC = M_TOT // N_CORES
    xf = np.ascontiguousarray(x.reshape(M_TOT, E))

    w1p = np.ascontiguousarray(
        w1.reshape(E // P, P, H // P, P).transpose(2, 1, 0, 3)
    ).astype(NP_BF16)
    w2p = np.ascontiguousarray(
        w2.reshape(H // P, P, E2 // P, P).transpose(2, 1, 0, 3)
    ).astype(NP_BF16)
    b1p = np.ascontiguousarray(b1.reshape(H // P, P).T)
    b2p = np.ascontiguousarray(b2.reshape(E2 // P, P).T)

    in_maps = []
    for i in range(N_CORES):
        xTi = np.ascontiguousarray(xf[i * MC : (i + 1) * MC].T).astype(NP_BF16)
        in_maps.append(
            {"xT": xTi, "w1p": w1p, "b1p": b1p, "w2p": w2p, "b2p": b2p}
        )
    return in_maps


def unpack_outputs(results, batch_shape=(4, 2048), E2=1024):
    M_TOT = batch_shape[0] * batch_shape[1]
    MC = M_TOT // N_CORES
    out = np.empty((M_TOT, E2), dtype=np.float32)
    for i in range(N_CORES):
        o = results[i]["outT"]  # [E2B, P, MC]
        out[i * MC : (i + 1) * MC] = o.transpose(2, 0, 1).reshape(MC, E2)
    return out.reshape(*batch_shape, E2)


_NC_CACHE = {}


def _get_nc():
    if "nc" not in _NC_CACHE:
        _NC_CACHE["nc"] = build_nc()
    return _NC_CACHE["nc"]


def kernel(x, w1, b1, w2, b2):
    nc = _get_nc()
    in_maps = pack_inputs(
        np.asarray(x, dtype=np.float32),
        np.asarray(w1, dtype=np.float32),
        np.asarray(b1, dtype=np.float32),
        np.asarray(w2, dtype=np.float32),
        np.asarray(b2, dtype=np.float32),
    )
    res = run_bass_kernel_spmd(nc, in_maps, core_ids=list(range(N_CORES))).results
    return unpack_outputs(res, batch_shape=(x.shape[0], x.shape[1]), E2=w2.shape[1])
